# revision 4
# baseline (speedup 1.0000x reference)
"""Trainium2 Bass kernel for CtaPostAttnMixer (4-step 1D heat-diffusion
stencil along seq with fixed endpoints) on x[4, 8192, 1024] f32.

Strategy
--------
The 4 diffusion steps compose into ONE banded linear operator along seq
(bandwidth 4, i.e. 9 taps), with boundary-modified rows only at the first
and last 4 positions of the sequence.  So the whole op is a single pass:

    out[l] = sum_{t=-4..4} K4[t] * x[l+t]     (interior)

computed per-core as dense [120 out-rows x 128 window-rows] matmuls on the
tensor engine: rows of x live on SBUF partitions, channels (d=1024) stream
as the matmul free dim.  One matmul pair (2 x N=512 fp32) per output tile.

Sharding: 8 cores = 4 batches x 2 sequence halves.  Each core gets its
half's rows plus a 4-row halo on each side ([4104, 1024] padded at global
sequence ends) and produces [4096, 1024].  Per-core boundary handling is
pure data: each core receives its own stack of 3 [128, 120] operator
matrices (first-tile / interior / tail-tile).
"""

import numpy as np

ALPHA, STEPS = 0.1, 4
B, L, D = 4, 8192, 1024
HALF = L // 2          # 4096 output rows per core
NIN = HALF + 8         # 4104 input rows per core (4-row halo each side)
MTILE = 120            # out rows per full tile (window 128 - 2*4 halo)
NT_FULL = 34           # full tiles: 34 * 120 = 4080 rows
TAIL_S = 3976          # tail window start (local input coords)
TAIL_M = 16            # tail out rows: 4080..4096
NHALF = D // 2         # matmul free-dim chunk (fp32 max 512)
N_CORES = 8
SLABS = [(0, 8), (8, 8), (16, 8), (24, 8), (32, 2)]  # (first tile J0, count)


def _taps():
    k1 = np.array([ALPHA, 1 - 2 * ALPHA, ALPHA], dtype=np.float64)
    k = k1.copy()
    for _ in range(STEPS - 1):
        k = np.convolve(k, k1)
    return k  # 9 taps, index 0..8 <-> offset -4..4


def _boundary_T4(n=256):
    T = np.zeros((n, n))
    T[0, 0] = 1.0
    T[-1, -1] = 1.0
    for i in range(1, n - 1):
        T[i, i - 1] = ALPHA
        T[i, i] = 1 - 2 * ALPHA
        T[i, i + 1] = ALPHA
    return np.linalg.matrix_power(T, STEPS)


def _build_mats(half):
    """Per-core operator stack [3, 128, MTILE] in lhsT layout
    (lhsT[window_row, out_row]); index 0 = tile J=0, 1 = interior,
    2 = tail tile."""
    K4 = _taps()
    T4 = _boundary_T4()
    n = T4.shape[0]

    A_mid = np.zeros((MTILE, 128))
    for r in range(MTILE):
        A_mid[r, r:r + 9] = K4

    if half == 0:
        # tile 0 holds the global sequence start: local window row p is
        # global row p-4 (p<4 is padding; boundary operator has no taps
        # there, so those columns stay zero).
        A_first = np.zeros((MTILE, 128))
        for r in range(MTILE):
            lo = max(4, r)          # K4 support [r, r+8] but global >= 0
            A_first[r, lo:r + 9] = T4[r, lo - 4:r + 5]
        A_tail = np.zeros((MTILE, 128))
        for r in range(TAIL_M):
            A_tail[r, 104 + r:104 + r + 9] = K4
    else:
        A_first = A_mid
        # tail holds the global sequence end: out global rows 8176..8191
        # <-> segment rows n-16+r; window col p <-> segment col n-124+p
        # (p >= 124 is padding; zero there).
        A_tail = np.zeros((MTILE, 128))
        for r in range(TAIL_M):
            seg = T4[n - 16 + r]
            A_tail[r, :124] = seg[n - 124:n]
    stack = np.stack([A_first, A_mid, A_tail])          # [3, MTILE, 128]
    return np.ascontiguousarray(stack.transpose(0, 2, 1)).astype(np.float32)


def _split_multi_waits(nc):
    """This container's walrus accepts only ONE sync-wait per instruction,
    but Tile liberally attaches several (e.g. a matmul waiting on two DMA
    sems, or the kernel-tail Drain waiting on everything).  Engine streams
    execute in order, so hoisting extra waits onto single-wait NoOps placed
    immediately before the instruction is semantics-preserving."""
    import bass_rust

    ctr = 0
    for f in nc.m.functions:
        for blk in f.blocks:
            new = []
            for inst in blk.instructions:
                si = inst.sync_info
                if si is not None and len(si.on_wait) > 1:
                    waits = list(si.on_wait)
                    for w in waits[:-1]:
                        nop = bass_rust.InstNoOp(
                            name=f"wsplit_{ctr}", ins=[], outs=[],
                            engine=inst.engine,
                        )
                        ctr += 1
                        nop.sync_info = bass_rust.SyncInfo(
                            on_wait=[w], on_update=[]
                        )
                        new.append(nop)
                    inst.sync_info = bass_rust.SyncInfo(
                        on_wait=[waits[-1]], on_update=list(si.on_update)
                    )
                new.append(inst)
            blk.instructions = new


_PROGRAM = None


def _build_program():
    import concourse.bass as bass
    import concourse.mybir as mybir
    from concourse.tile import TileContext

    nc = bass.Bass("TRN2", target_bir_lowering=False, debug=False,
                   num_devices=N_CORES)
    f32 = mybir.dt.float32
    xs = nc.dram_tensor("xs", [NIN, D], f32, kind="ExternalInput").ap()
    mats = nc.dram_tensor("mats", [3, 128, MTILE], f32,
                          kind="ExternalInput").ap()
    ys = nc.dram_tensor("ys", [HALF, D], f32, kind="ExternalOutput").ap()

    with TileContext(nc) as tc:
        with (
            tc.tile_pool(name="consts", bufs=1) as const_pool,
            tc.tile_pool(name="inp", bufs=2) as in_pool,
            tc.tile_pool(name="outp", bufs=2) as out_pool,
            tc.tile_pool(name="tailp", bufs=1) as tail_pool,
            tc.tile_pool(name="psum", bufs=4, space="PSUM") as psum_pool,
        ):
            mats_sb = const_pool.tile([128, 3, MTILE], f32)
            nc.sync.dma_start(out=mats_sb[:], in_=mats.rearrange("m k p -> k m p"))

            for J0, C in SLABS:
                in_slab = in_pool.tile([128, 8, D], f32, tag="in_slab")
                # overlapping windows: window J starts at row 120*J, spans
                # 128 rows -> custom AP [part(row) step D x128,
                # window step 120*D xC, elem step 1 xD]
                src = bass.AP(
                    tensor=xs.tensor,
                    offset=MTILE * J0 * D,
                    ap=[[D, 128], [MTILE * D, C], [1, D]],
                )
                nc.sync.dma_start(out=in_slab[:, :C, :], in_=src)

                out_slab = out_pool.tile([MTILE, 8, D], f32, tag="out_slab")
                for c in range(C):
                    J = J0 + c
                    midx = 0 if J == 0 else 1
                    ps = psum_pool.tile([MTILE, D], f32, tag="ps")
                    for h in range(2):
                        nc.tensor.matmul(
                            ps[:, h * NHALF:(h + 1) * NHALF],
                            mats_sb[:, midx, :],
                            in_slab[:, c, h * NHALF:(h + 1) * NHALF],
                            start=True, stop=True,
                        )
                    nc.vector.tensor_copy(out=out_slab[:, c, :], in_=ps[:])
                nc.sync.dma_start(
                    out=ys[MTILE * J0:MTILE * (J0 + C)].rearrange(
                        "(c p) d -> p c d", p=MTILE),
                    in_=out_slab[:, :C, :],
                )

            # tail: out rows 4080..4096 from window [3976, 4104)
            tail_in = tail_pool.tile([128, D], f32, tag="tail_in")
            nc.sync.dma_start(out=tail_in[:], in_=xs[TAIL_S:TAIL_S + 128])
            ps = psum_pool.tile([MTILE, D], f32, tag="ps")
            for h in range(2):
                nc.tensor.matmul(
                    ps[:, h * NHALF:(h + 1) * NHALF],
                    mats_sb[:, 2, :],
                    tail_in[:, h * NHALF:(h + 1) * NHALF],
                    start=True, stop=True,
                )
            tail_out = tail_pool.tile([TAIL_M, D], f32, tag="tail_out")
            nc.vector.tensor_copy(out=tail_out[:], in_=ps[:TAIL_M, :])
            nc.sync.dma_start(out=ys[NT_FULL * MTILE:HALF], in_=tail_out[:])

    _split_multi_waits(nc)
    return nc


def kernel(x):
    global _PROGRAM
    from concourse import bass_utils

    x = np.ascontiguousarray(np.asarray(x), dtype=np.float32)
    assert x.shape == (B, L, D), x.shape

    mats_by_half = [_build_mats(0), _build_mats(1)]
    in_maps = []
    for k in range(N_CORES):
        b, half = k // 2, k % 2
        l0 = HALF * half
        xs = np.zeros((NIN, D), np.float32)
        lo, hi = l0 - 4, l0 + HALF + 4
        s_lo, s_hi = max(lo, 0), min(hi, L)
        xs[s_lo - lo:s_hi - lo] = x[b, s_lo:s_hi]
        in_maps.append({"xs": xs, "mats": mats_by_half[half]})

    if _PROGRAM is None:
        _PROGRAM = _build_program()

    res = bass_utils.run_bass_kernel_spmd(
        _PROGRAM, in_maps, core_ids=list(range(N_CORES)), trace=False
    )

    out = np.empty((B, L, D), np.float32)
    for k in range(N_CORES):
        b, half = k // 2, k % 2
        out[b, HALF * half:HALF * (half + 1)] = res.results[k]["ys"]
    return out


# revision 8
# speedup vs baseline: 1.1226x; 1.1226x over previous
"""Trainium2 Bass kernel for CtaPostAttnMixer (4-step 1D heat-diffusion
stencil along seq with fixed endpoints) on x[4, 8192, 1024] f32.

Strategy
--------
The 4 diffusion steps compose into ONE banded linear operator along seq
(bandwidth 4, i.e. 9 taps), with boundary-modified rows only at the first
and last 4 positions of the sequence.  So the whole op is a single pass:

    out[l] = sum_{t=-4..4} K4[t] * x[l+t]     (interior)

computed per-core as dense [120 out-rows x 128 window-rows] matmuls on the
tensor engine: rows of x live on SBUF partitions, channels (d=1024) stream
as the matmul free dim.  One matmul pair (2 x N=512 fp32) per output tile.

Sharding: 8 cores = 4 batches x 2 sequence halves.  Each core gets its
half's rows plus a 4-row halo on each side ([4104, 1024] padded at global
sequence ends) and produces [4096, 1024].  Per-core boundary handling is
pure data: each core receives its own stack of 3 [128, 120] operator
matrices (first-tile / interior / tail-tile).
"""

import numpy as np

ALPHA, STEPS = 0.1, 4
B, L, D = 4, 8192, 1024
HALF = L // 2          # 4096 output rows per core
NIN = HALF + 8         # 4104 input rows per core (4-row halo each side)
MTILE = 120            # out rows per full tile (window 128 - 2*4 halo)
NT_FULL = 34           # full tiles: 34 * 120 = 4080 rows
TAIL_S = 3976          # tail window start (local input coords)
TAIL_M = 16            # tail out rows: 4080..4096
NHALF = D // 2         # matmul free-dim chunk (fp32 max 512)
N_CORES = 8
SLABS = [(0, 8), (8, 8), (16, 8), (24, 8), (32, 2)]  # (first tile J0, count)


def _taps():
    k1 = np.array([ALPHA, 1 - 2 * ALPHA, ALPHA], dtype=np.float64)
    k = k1.copy()
    for _ in range(STEPS - 1):
        k = np.convolve(k, k1)
    return k  # 9 taps, index 0..8 <-> offset -4..4


def _boundary_T4(n=256):
    T = np.zeros((n, n))
    T[0, 0] = 1.0
    T[-1, -1] = 1.0
    for i in range(1, n - 1):
        T[i, i - 1] = ALPHA
        T[i, i] = 1 - 2 * ALPHA
        T[i, i + 1] = ALPHA
    return np.linalg.matrix_power(T, STEPS)


def _build_mats(half):
    """Per-core operator stack [3, 128, MTILE] in lhsT layout
    (lhsT[window_row, out_row]); index 0 = tile J=0, 1 = interior,
    2 = tail tile."""
    K4 = _taps()
    T4 = _boundary_T4()
    n = T4.shape[0]

    A_mid = np.zeros((MTILE, 128))
    for r in range(MTILE):
        A_mid[r, r:r + 9] = K4

    if half == 0:
        # tile 0 holds the global sequence start: local window row p is
        # global row p-4 (p<4 is padding; boundary operator has no taps
        # there, so those columns stay zero).
        A_first = np.zeros((MTILE, 128))
        for r in range(MTILE):
            lo = max(4, r)          # K4 support [r, r+8] but global >= 0
            A_first[r, lo:r + 9] = T4[r, lo - 4:r + 5]
        A_tail = np.zeros((MTILE, 128))
        for r in range(TAIL_M):
            A_tail[r, 104 + r:104 + r + 9] = K4
    else:
        A_first = A_mid
        # tail holds the global sequence end: out global rows 8176..8191
        # <-> segment rows n-16+r; window col p <-> segment col n-124+p
        # (p >= 124 is padding; zero there).
        A_tail = np.zeros((MTILE, 128))
        for r in range(TAIL_M):
            seg = T4[n - 16 + r]
            A_tail[r, :124] = seg[n - 124:n]
    stack = np.stack([A_first, A_mid, A_tail])          # [3, MTILE, 128]
    return np.ascontiguousarray(stack.transpose(0, 2, 1)).astype(np.float32)


def _split_multi_waits(nc):
    """This container's walrus accepts only ONE sync-wait per instruction,
    but Tile liberally attaches several (e.g. a matmul waiting on two DMA
    sems, or the kernel-tail Drain waiting on everything).  Engine streams
    execute in order, so hoisting extra waits onto single-wait NoOps placed
    immediately before the instruction is semantics-preserving."""
    import bass_rust

    ctr = 0
    for f in nc.m.functions:
        for blk in f.blocks:
            new = []
            for inst in blk.instructions:
                si = inst.sync_info
                if si is not None and len(si.on_wait) > 1:
                    waits = list(si.on_wait)
                    for w in waits[:-1]:
                        nop = bass_rust.InstNoOp(
                            name=f"wsplit_{ctr}", ins=[], outs=[],
                            engine=inst.engine,
                        )
                        ctr += 1
                        nop.sync_info = bass_rust.SyncInfo(
                            on_wait=[w], on_update=[]
                        )
                        new.append(nop)
                    inst.sync_info = bass_rust.SyncInfo(
                        on_wait=[waits[-1]], on_update=list(si.on_update)
                    )
                new.append(inst)
            blk.instructions = new


_PROGRAM = None


def _build_program():
    import concourse.bass as bass
    import concourse.mybir as mybir
    from concourse.tile import TileContext

    nc = bass.Bass("TRN2", target_bir_lowering=False, debug=False,
                   num_devices=N_CORES)
    f32 = mybir.dt.float32
    xs = nc.dram_tensor("xs", [NIN, D], f32, kind="ExternalInput").ap()
    mats = nc.dram_tensor("mats", [3, 128, MTILE], f32,
                          kind="ExternalInput").ap()
    ys = nc.dram_tensor("ys", [HALF, D], f32, kind="ExternalOutput").ap()

    with TileContext(nc) as tc:
        with (
            tc.tile_pool(name="consts", bufs=1) as const_pool,
            tc.tile_pool(name="inp", bufs=3) as in_pool,
            tc.tile_pool(name="outp", bufs=3) as out_pool,
            tc.tile_pool(name="tailp", bufs=1) as tail_pool,
            tc.tile_pool(name="psum", bufs=4, space="PSUM") as psum_pool,
        ):
            mats_sb = const_pool.tile([128, 3, MTILE], f32)
            nc.scalar.dma_start(out=mats_sb[:], in_=mats.rearrange("m k p -> k m p"))

            for J0, C in SLABS:
                in_slab = in_pool.tile([128, 8, D], f32, tag="in_slab")
                # overlapping windows: window J starts at row 120*J, spans
                # 128 rows -> custom AP [part(row) step D x128,
                # window step 120*D xC, elem step 1 xD]
                src = bass.AP(
                    tensor=xs.tensor,
                    offset=MTILE * J0 * D,
                    ap=[[D, 128], [MTILE * D, C], [1, D]],
                )
                nc.sync.dma_start(out=in_slab[:, :C, :], in_=src)

                out_slab = out_pool.tile([MTILE, 8, D], f32, tag="out_slab")
                for c in range(C):
                    J = J0 + c
                    midx = 0 if J == 0 else 1
                    ps = psum_pool.tile([MTILE, D], f32, tag="ps")
                    for h in range(2):
                        nc.tensor.matmul(
                            ps[:, h * NHALF:(h + 1) * NHALF],
                            mats_sb[:, midx, :],
                            in_slab[:, c, h * NHALF:(h + 1) * NHALF],
                            start=True, stop=True,
                        )
                    nc.vector.tensor_copy(out=out_slab[:, c, :], in_=ps[:])
                # output stream on the ACT HWDGE ring: an out-DMA waiting on
                # copies must not block descriptor-gen of later input loads
                # (which use the SP ring).
                nc.scalar.dma_start(
                    out=ys[MTILE * J0:MTILE * (J0 + C)].rearrange(
                        "(c p) d -> p c d", p=MTILE),
                    in_=out_slab[:, :C, :],
                )

            # tail: out rows 4080..4096 from window [3976, 4104)
            tail_in = tail_pool.tile([128, D], f32, tag="tail_in")
            nc.sync.dma_start(out=tail_in[:], in_=xs[TAIL_S:TAIL_S + 128])
            ps = psum_pool.tile([MTILE, D], f32, tag="ps")
            for h in range(2):
                nc.tensor.matmul(
                    ps[:, h * NHALF:(h + 1) * NHALF],
                    mats_sb[:, 2, :],
                    tail_in[:, h * NHALF:(h + 1) * NHALF],
                    start=True, stop=True,
                )
            tail_out = tail_pool.tile([TAIL_M, D], f32, tag="tail_out")
            nc.vector.tensor_copy(out=tail_out[:], in_=ps[:TAIL_M, :])
            nc.scalar.dma_start(out=ys[NT_FULL * MTILE:HALF], in_=tail_out[:])

    _split_multi_waits(nc)
    return nc


def kernel(x):
    global _PROGRAM
    from concourse import bass_utils

    x = np.ascontiguousarray(np.asarray(x), dtype=np.float32)
    assert x.shape == (B, L, D), x.shape

    mats_by_half = [_build_mats(0), _build_mats(1)]
    in_maps = []
    for k in range(N_CORES):
        b, half = k // 2, k % 2
        l0 = HALF * half
        xs = np.zeros((NIN, D), np.float32)
        lo, hi = l0 - 4, l0 + HALF + 4
        s_lo, s_hi = max(lo, 0), min(hi, L)
        xs[s_lo - lo:s_hi - lo] = x[b, s_lo:s_hi]
        in_maps.append({"xs": xs, "mats": mats_by_half[half]})

    if _PROGRAM is None:
        _PROGRAM = _build_program()

    res = bass_utils.run_bass_kernel_spmd(
        _PROGRAM, in_maps, core_ids=list(range(N_CORES)), trace=False
    )

    out = np.empty((B, L, D), np.float32)
    for k in range(N_CORES):
        b, half = k // 2, k % 2
        out[b, HALF * half:HALF * (half + 1)] = res.results[k]["ys"]
    return out


# revision 13
# speedup vs baseline: 1.3583x; 1.2100x over previous
"""Trainium2 Bass kernel for CtaPostAttnMixer (4-step 1D heat-diffusion
stencil along seq with fixed endpoints) on x[4, 8192, 1024] f32.

Strategy
--------
The 4 diffusion steps compose into ONE banded linear operator along seq
(bandwidth 4, i.e. 9 taps), with boundary-modified rows only at the first
and last 4 positions of the sequence.  So the whole op is a single pass:

    out[l] = sum_{t=-4..4} K4[t] * x[l+t]     (interior)

computed per-core as dense [120 out-rows x 128 window-rows] matmuls on the
tensor engine: rows of x live on SBUF partitions, channels (d=1024) stream
as the matmul free dim.  One matmul pair (2 x N=512 fp32) per output tile.

Sharding: 8 cores = 4 batches x 2 sequence halves.  Each core gets its
half's rows plus a 4-row halo on each side ([4104, 1024] padded at global
sequence ends) and produces [4096, 1024].  Per-core boundary handling is
pure data: each core receives its own stack of 3 [128, 120] operator
matrices (first-tile / interior / tail-tile).
"""

import numpy as np

ALPHA, STEPS = 0.1, 4
B, L, D = 4, 8192, 1024
HALF = L // 2          # 4096 output rows per core
NIN = HALF + 8         # 4104 input rows per core (4-row halo each side)
MTILE = 120            # out rows per full tile (window 128 - 2*4 halo)
NT_FULL = 34           # full tiles: 34 * 120 = 4080 rows
TAIL_S = 3976          # tail window start (local input coords)
TAIL_M = 16            # tail out rows: 4080..4096
NHALF = D // 2         # matmul free-dim chunk (fp32 max 512)
N_CORES = 8
SLABS = [(0, 4), (4, 4), (8, 4), (12, 4), (16, 4), (20, 4), (24, 4),
         (28, 4), (32, 2)]  # (first tile J0, count)


def _taps():
    k1 = np.array([ALPHA, 1 - 2 * ALPHA, ALPHA], dtype=np.float64)
    k = k1.copy()
    for _ in range(STEPS - 1):
        k = np.convolve(k, k1)
    return k  # 9 taps, index 0..8 <-> offset -4..4


def _boundary_T4(n=256):
    T = np.zeros((n, n))
    T[0, 0] = 1.0
    T[-1, -1] = 1.0
    for i in range(1, n - 1):
        T[i, i - 1] = ALPHA
        T[i, i] = 1 - 2 * ALPHA
        T[i, i + 1] = ALPHA
    return np.linalg.matrix_power(T, STEPS)


def _build_mats(half):
    """Per-core operator stack [3, 128, MTILE] in lhsT layout
    (lhsT[window_row, out_row]); index 0 = tile J=0, 1 = interior,
    2 = tail tile."""
    K4 = _taps()
    T4 = _boundary_T4()
    n = T4.shape[0]

    A_mid = np.zeros((MTILE, 128))
    for r in range(MTILE):
        A_mid[r, r:r + 9] = K4

    if half == 0:
        # tile 0 holds the global sequence start: local window row p is
        # global row p-4 (p<4 is padding; boundary operator has no taps
        # there, so those columns stay zero).
        A_first = np.zeros((MTILE, 128))
        for r in range(MTILE):
            lo = max(4, r)          # K4 support [r, r+8] but global >= 0
            A_first[r, lo:r + 9] = T4[r, lo - 4:r + 5]
        A_tail = np.zeros((MTILE, 128))
        for r in range(TAIL_M):
            A_tail[r, 104 + r:104 + r + 9] = K4
    else:
        A_first = A_mid
        # tail holds the global sequence end: out global rows 8176..8191
        # <-> segment rows n-16+r; window col p <-> segment col n-124+p
        # (p >= 124 is padding; zero there).
        A_tail = np.zeros((MTILE, 128))
        for r in range(TAIL_M):
            seg = T4[n - 16 + r]
            A_tail[r, :124] = seg[n - 124:n]
    stack = np.stack([A_first, A_mid, A_tail])          # [3, MTILE, 128]
    return np.ascontiguousarray(stack.transpose(0, 2, 1)).astype(np.float32)


def _split_multi_waits(nc):
    """This container's walrus accepts only ONE sync-wait per instruction,
    but Tile liberally attaches several (e.g. a matmul waiting on two DMA
    sems, or the kernel-tail Drain waiting on everything).  Engine streams
    execute in order, so hoisting extra waits onto single-wait NoOps placed
    immediately before the instruction is semantics-preserving."""
    import bass_rust

    ctr = 0
    for f in nc.m.functions:
        for blk in f.blocks:
            new = []
            for inst in blk.instructions:
                si = inst.sync_info
                if si is not None and len(si.on_wait) > 1:
                    waits = list(si.on_wait)
                    for w in waits[:-1]:
                        nop = bass_rust.InstNoOp(
                            name=f"wsplit_{ctr}", ins=[], outs=[],
                            engine=inst.engine,
                        )
                        ctr += 1
                        nop.sync_info = bass_rust.SyncInfo(
                            on_wait=[w], on_update=[]
                        )
                        new.append(nop)
                    inst.sync_info = bass_rust.SyncInfo(
                        on_wait=[waits[-1]], on_update=list(si.on_update)
                    )
                new.append(inst)
            blk.instructions = new


_PROGRAM = None


def _build_program():
    import concourse.bass as bass
    import concourse.mybir as mybir
    from concourse.tile import TileContext

    nc = bass.Bass("TRN2", target_bir_lowering=False, debug=False,
                   num_devices=N_CORES)
    f32 = mybir.dt.float32
    xs = nc.dram_tensor("xs", [NIN, D], f32, kind="ExternalInput").ap()
    mats = nc.dram_tensor("mats", [3, 128, MTILE], f32,
                          kind="ExternalInput").ap()
    ys = nc.dram_tensor("ys", [HALF, D], f32, kind="ExternalOutput").ap()

    with TileContext(nc) as tc:
        with (
            tc.tile_pool(name="consts", bufs=1) as const_pool,
            tc.tile_pool(name="inp", bufs=4) as in_pool,
            tc.tile_pool(name="outp", bufs=4) as out_pool,
            tc.tile_pool(name="tailp", bufs=1) as tail_pool,
            tc.tile_pool(name="psum", bufs=4, space="PSUM") as psum_pool,
        ):
            mats_sb = const_pool.tile([128, 3, MTILE], f32)
            nc.scalar.dma_start(out=mats_sb[:], in_=mats.rearrange("m k p -> k m p"))

            # tail first so the kernel doesn't end on this serial
            # load->matmul->copy->store chain
            tail_in = tail_pool.tile([128, D], f32, tag="tail_in")
            nc.sync.dma_start(out=tail_in[:], in_=xs[TAIL_S:TAIL_S + 128])
            ps = psum_pool.tile([MTILE, D], f32, tag="ps")
            for h in range(2):
                nc.tensor.matmul(
                    ps[:, h * NHALF:(h + 1) * NHALF],
                    mats_sb[:, 2, :],
                    tail_in[:, h * NHALF:(h + 1) * NHALF],
                    start=True, stop=True,
                )
            tail_out = tail_pool.tile([TAIL_M, D], f32, tag="tail_out")
            nc.vector.tensor_copy(out=tail_out[:], in_=ps[:TAIL_M, :])
            nc.scalar.dma_start(out=ys[NT_FULL * MTILE:HALF], in_=tail_out[:])

            for J0, C in SLABS:
                in_slab = in_pool.tile([128, 4, D], f32, tag="in_slab")
                # overlapping windows: window J starts at row 120*J, spans
                # 128 rows -> custom AP [part(row) step D x128,
                # window step 120*D xC, elem step 1 xD]
                src = bass.AP(
                    tensor=xs.tensor,
                    offset=MTILE * J0 * D,
                    ap=[[D, 128], [MTILE * D, C], [1, D]],
                )
                nc.sync.dma_start(out=in_slab[:, :C, :], in_=src)

                out_slab = out_pool.tile([MTILE, 4, D], f32, tag="out_slab")
                for c in range(C):
                    J = J0 + c
                    midx = 0 if J == 0 else 1
                    ps = psum_pool.tile([MTILE, D], f32, tag="ps")
                    for h in range(2):
                        nc.tensor.matmul(
                            ps[:, h * NHALF:(h + 1) * NHALF],
                            mats_sb[:, midx, :],
                            in_slab[:, c, h * NHALF:(h + 1) * NHALF],
                            start=True, stop=True,
                        )
                    nc.vector.tensor_copy(out=out_slab[:, c, :], in_=ps[:])
                # output stream on the ACT HWDGE ring: an out-DMA waiting on
                # copies must not block descriptor-gen of later input loads
                # (which use the SP ring).
                nc.scalar.dma_start(
                    out=ys[MTILE * J0:MTILE * (J0 + C)].rearrange(
                        "(c p) d -> p c d", p=MTILE),
                    in_=out_slab[:, :C, :],
                )

    _split_multi_waits(nc)
    return nc


def kernel(x):
    global _PROGRAM
    from concourse import bass_utils

    x = np.ascontiguousarray(np.asarray(x), dtype=np.float32)
    assert x.shape == (B, L, D), x.shape

    mats_by_half = [_build_mats(0), _build_mats(1)]
    in_maps = []
    for k in range(N_CORES):
        b, half = k // 2, k % 2
        l0 = HALF * half
        xs = np.zeros((NIN, D), np.float32)
        lo, hi = l0 - 4, l0 + HALF + 4
        s_lo, s_hi = max(lo, 0), min(hi, L)
        xs[s_lo - lo:s_hi - lo] = x[b, s_lo:s_hi]
        in_maps.append({"xs": xs, "mats": mats_by_half[half]})

    if _PROGRAM is None:
        _PROGRAM = _build_program()

    res = bass_utils.run_bass_kernel_spmd(
        _PROGRAM, in_maps, core_ids=list(range(N_CORES)), trace=False
    )

    out = np.empty((B, L, D), np.float32)
    for k in range(N_CORES):
        b, half = k // 2, k % 2
        out[b, HALF * half:HALF * (half + 1)] = res.results[k]["ys"]
    return out
